# revision 26
# baseline (speedup 1.0000x reference)
"""Trainium2 Bass kernel for nn_MetaController (GRU + gated scan + hypernet decoder).

Self-contained: kernel(**inputs) -> np.ndarray [2,1024,1024] float32.

Two SPMD programs on 8 NeuronCores:
  P1: GRU solved by fixed-point iteration (scan-variant): 7 sweeps total
      (sweep 0 elementwise, 6 sweeps with the h->3h matmul batched over all
      2048 tokens). Each sweep: hp = H_prev_shifted @ w_hh^T (8-way
      channel-sharded, bf16), gates on ACT/DVE, z-diagonal recurrence solved
      exactly via DVE tensor_tensor_scan, then the 128-channel H chunk is
      broadcast to all cores via remote SBUF DMA. Converges to the bf16
      floor (~1e-4 end-to-end) in 6 matmul sweeps. Emits per-core partial
      beta projections; host sums + applies sigmoid.
  P2: gated associative scan via DVE tensor_tensor_scan, decoder mm1 (gelu)
      replicated, 16384-row w1-half of the decoder output tensor-parallel in
      r-major row order so the low-rank contraction sum_r w1*(w2 row-sums)
      becomes 16 broadcast-multiply-accumulates. The w2-half collapses to 16
      columns via host-presummed W2s.
"""
import sys
sys.path.insert(0, '/opt/trn_rl_repo')
import numpy as np
import ml_dtypes
import concourse.bass as bass
import concourse.mybir as mybir
from concourse.bass import ds
from concourse import library_config, library_overlay, bacc
from concourse.tile import TileContext
from concourse.bass_utils import run_bass_kernel_spmd

F32 = mybir.dt.float32
BF16 = mybir.dt.bfloat16
I32 = mybir.dt.int32
AF = mybir.ActivationFunctionType

B, N, D, R, H = 2, 1024, 1024, 16, 2048
P = 128
T = B * N            # 2048 tokens, order b*N + n
TPAD = N + 1         # per-chain padded length (zero col at chain start)
BT = B * TPAD        # 2050
NSWEEP = 2           # matmul sweeps (plus sweep 0 without matmul)
CH = 512             # token f-chunk
NF = T // CH         # 4 f-chunks (2 per chain)
NM = 24              # P1 m-tiles: m = g*8 + k_out
XR = 8               # P1 xin ring slots
XO = 4               # P1 xout ring slots


# ------------------------------------------------------------------ P1 (GRU)
def p1r_host_prep(inputs):
    w_ih = np.asarray(inputs["gru_w_ih"], np.float32)
    w_hh = np.asarray(inputs["gru_w_hh"], np.float32)
    b_ih = np.asarray(inputs["gru_b_ih"], np.float32)
    b_hh = np.asarray(inputs["gru_b_hh"], np.float32)
    beta_w = np.asarray(inputs["beta_w"], np.float32)
    lat = np.asarray(inputs["latent"], np.float32)
    bf = ml_dtypes.bfloat16

    latT = np.ascontiguousarray(lat.transpose(2, 0, 1).reshape(D, T))
    sgn = np.array([1.0, -1.0, 1.0], np.float32)
    wih = np.concatenate([sgn[g] * w_ih[g * D:(g + 1) * D] for g in range(3)], 0)
    whh = np.concatenate([sgn[g] * w_hh[g * D:(g + 1) * D] for g in range(3)], 0)
    assert not np.any(b_hh[2 * D:]), "b_hh n-gate must be zero"
    bias = np.concatenate([(b_ih[:2 * D] + b_hh[:2 * D]) * np.repeat(sgn[:2], D),
                           b_ih[2 * D:]])  # [3D]; m-block col = bias[m*128:...]
    bias24 = bias.reshape(NM, P).T                    # [P, 24]
    biasz_neg = -bias24[:, 8:16]                      # [P, 8] (z chunks, negated)
    bias_pc = np.concatenate([bias24, biasz_neg], 1)  # [P, 32]
    return {
        "latT_tb": latT.astype(bf),
        "wih_l": np.ascontiguousarray(wih.T).astype(bf),   # [D, 3D] lhsT
        "whh_l": np.ascontiguousarray(whh.T).astype(bf),
        "bias_pc": np.ascontiguousarray(bias_pc, np.float32),
        "bw_pc": np.ascontiguousarray(beta_w.reshape(8, P).T).astype(bf),  # [P,8]
    }


def p1r_build(nc):
    latT_tb = nc.declare_dram_parameter("latT_tb", [D, T], BF16, isOutput=False)
    wih_l = nc.declare_dram_parameter("wih_l", [D, 3 * D], BF16, isOutput=False)
    whh_l = nc.declare_dram_parameter("whh_l", [D, 3 * D], BF16, isOutput=False)
    bias_pc = nc.declare_dram_parameter("bias_pc", [P, 32], F32, isOutput=False)
    bw_pc = nc.declare_dram_parameter("bw_pc", [P, 8], BF16, isOutput=False)
    betap = nc.declare_dram_parameter("betap", [1, T], F32, isOutput=True)
    xp_d = nc.dram_tensor("xp_d", [NM * P, T], BF16)

    from contextlib import ExitStack
    with ExitStack() as ctx:
        def sbuf(name, shape, dtype):
            return ctx.enter_context(nc.sbuf_tensor(name, shape, dtype))

        def sem(name):
            return ctx.enter_context(nc.semaphore(name))

        latT_s = sbuf("latT_s", [P, 8 * T], BF16)
        w_s = sbuf("w_s", [P, 192 * P], BF16)       # wih for phase 0, whh after
        hb0 = sbuf("hb0", [P, 8 * BT], BF16)
        hb1 = sbuf("hb1", [P, 8 * BT], BF16)
        xin_s = sbuf("xin_s", [P, XR * CH], BF16)
        xout_s = sbuf("xout_s", [P, XO * CH], BF16)
        bias_s = sbuf("bias_s", [P, 32], F32)
        bw_s = sbuf("bw_s", [P, 8], BF16)
        tr_s = sbuf("tr_s", [P, 2 * CH], F32)
        tz_s = sbuf("tz_s", [P, 2 * CH], F32)
        tn_s = sbuf("tn_s", [P, 2 * CH], F32)
        tn2_s = sbuf("tn2_s", [P, 2 * CH], F32)
        r_s = sbuf("r_s", [P, 2 * CH], BF16)
        zb_s = sbuf("zb_s", [P, 2 * CH], BF16)
        z_s = sbuf("z_s", [P, 2 * CH], BF16)
        n_s = sbuf("n_s", [P, 2 * CH], BF16)
        nz_s = sbuf("nz_s", [P, 2 * CH], BF16)
        betap_s = sbuf("betap_s", [1, T], F32)

        pg = [ctx.enter_context(nc.psum_tensor(f"pg{i}", [P, CH], F32))
              for i in range(6)]
        psb = ctx.enter_context(nc.psum_tensor("psb", [1, CH], F32))

        dma_sem = sem("dma_sem")
        s_pa = sem("s_pa")    # PE psum group done (cumulative)
        s_a0 = sem("s_a0")    # phase-0 psum fully consumed (ACT, 1/group)
        s_gc = sem("s_gc")    # sweep psum+xin consumed (DVE tr/tz/tn2)
        s_a1 = sem("s_a1")    # ACT r ready (sweeps, 1/(k,f))
        s_tn = sem("s_tn")    # DVE tn retired (self-sync)
        s_d2 = sem("s_d2")    # DVE tn2 ready
        s_a2 = sem("s_a2")    # ACT zb,z,n done (sweeps, 1/(k,f))
        s_nz = sem("s_nz")    # DVE nz retired
        s_hs = sem("s_hs")    # scan chunk done (memsets count 2)
        s_xo = sem("s_xo")    # ACT wrote xout slot
        s_xod = [sem(f"s_xod{s}") for s in range(XO)]  # per-slot xout->DRAM done
        s_xin = [sem(f"s_xin{s}") for s in range(XR)]  # per-slot xin landed
        s_whk = [sem(f"s_whk{k}") for k in range(8)]  # whh k_out block loaded
        s_wik = [sem(f"s_wik{k}") for k in range(8)]  # wih k_out block loaded
        s_latf = [sem(f"s_latf{f}") for f in range(NF)]  # latT f-chunk loaded
        s_pb = sem("s_pb")
        s_ab = sem("s_ab")

        lat3 = latT_s[:].rearrange("p (k c) -> p k c", k=8)
        wv = w_s[:].rearrange("p (k m c) -> p k m c", k=8, m=NM)
        hb = [hb0, hb1]

        def fcols(f):
            return slice(f * CH, (f + 1) * CH)

        def slot(kf, g):
            return (kf % 2) * 3 + g

        def hoff(f):  # data col offset of chunk f inside one padded 2050 block
            return (f // 2) * TPAD + 1 + (f % 2) * CH

        with nc.Block() as block:
            @block.sync
            def _(sync):
                for ff in range(NF):
                    sync.dma_start(out=lat3[:, :, fcols(ff)],
                                   in_=latT_tb[:, fcols(ff)]
                                   .rearrange("(k p) c -> p k c", p=P)
                                   ).then_inc(s_latf[ff], 16)
                for ko in range(8):
                    for g in range(3):
                        m = g * 8 + ko
                        sync.dma_start(out=wv[:, :, m, :],
                                       in_=wih_l[:, m * P:(m + 1) * P]
                                       .rearrange("(k p) c -> p k c", p=P)
                                       ).then_inc(s_wik[ko], 16)
                sync.dma_start(out=bias_s[:], in_=bias_pc[:, :]).then_inc(dma_sem, 16)
                sync.dma_start(out=bw_s[:], in_=bw_pc[:, :]).then_inc(dma_sem, 16)
                # xout -> DRAM as ACT produces tiles (96)
                xoc = 0
                for i in range(NM * NF):
                    kf, g = i // 3, i % 3
                    k_out, f = kf // NF, kf % NF
                    m = g * 8 + k_out
                    if g == 0:
                        sync.wait_ge(s_a0, i + 1)
                    else:
                        xoc += 1
                        sync.wait_ge(s_xo, xoc)
                    sync.dma_start(out=xp_d[m * P:(m + 1) * P, fcols(f)],
                                   in_=xout_s[:, (i % XO) * CH:(i % XO + 1) * CH]
                                   ).then_inc(s_xod[i % XO], 16)
                # reload w_s with whh per (g, k_out) block as phase-0 frees it
                for ko in range(8):
                    sync.wait_ge(s_pa, 12 * ko + 12)
                    for g in range(3):
                        m = g * 8 + ko
                        sync.dma_start(out=wv[:, :, m, :],
                                       in_=whh_l[:, m * P:(m + 1) * P]
                                       .rearrange("(k p) c -> p k c", p=P)
                                       ).then_inc(s_whk[ko], 16)
                # xin streaming for sweeps (tile index i_in = (j-1)*96+kf*3+t)
                for j in range(1, NSWEEP + 1):
                    for kf in range(32):
                        k_out, f = kf // NF, kf % NF
                        for t in range(3):          # 0=xr 1=xz 2=xn
                            i_in = (j - 1) * 96 + kf * 3 + t
                            m = t * 8 + k_out
                            i_out = kf * 3 + t
                            sync.wait_ge(s_xod[i_out % XO], 16 * (i_out // XO + 1))
                            if i_in >= XR:
                                # slot last held tile i_in-XR; wait its consumer
                                ip = i_in - XR
                                jp = ip // 96
                                kfp = (ip % 96) // 3
                                tp = ip % 3
                                if tp < 2:
                                    sync.wait_ge(s_gc, jp * 96 + kfp * 3 + tp + 1)
                                else:
                                    sync.wait_ge(s_d2, jp * 32 + kfp + 1)
                            sync.dma_start(out=xin_s[:, (i_in % XR) * CH:
                                                     (i_in % XR + 1) * CH],
                                           in_=xp_d[m * P:(m + 1) * P, fcols(f)]
                                           ).then_inc(s_xin[i_in % XR], 16)
                sync.wait_ge(s_ab, NF)
                sync.dma_start(out=betap[:, :], in_=betap_s[:]).then_inc(dma_sem, 16)
                sync.wait_ge(dma_sem, 48)

            @block.tensor
            def _(tensor):
                # phase 0: xp (= sweep-0 pre-acts) from latent
                for kf in range(32):
                    k_out, f = kf // NF, kf % NF
                    if kf < NF:
                        tensor.wait_ge(s_latf[f], 16)
                    if f == 0:
                        tensor.wait_ge(s_wik[k_out], 48)
                    for g in range(3):
                        q = kf * 3 + g
                        if q >= 6:
                            tensor.wait_ge(s_a0, q - 5)
                        pa = pg[slot(kf, g)][:, 0:CH]
                        m = g * 8 + k_out
                        for k in range(8):
                            mm = tensor.matmul(pa, wv[:, k, m, :],
                                               lat3[:, k, fcols(f)],
                                               start=(k == 0), stop=(k == 7))
                        mm.then_inc(s_pa, 1)
                # sweeps
                for j in range(1, NSWEEP + 1):
                    hbuf = hb[(j - 1) % 2]
                    for kf in range(32):
                        k_out, f = kf // NF, kf % NF
                        if j == 1 and kf % NF == 0:
                            tensor.wait_ge(s_whk[k_out], 48)
                        chain, half = f // 2, f % 2
                        st = chain * TPAD + half * CH  # shifted (t-1) cols
                        for g in range(3):
                            qs = (j - 1) * 96 + kf * 3 + g
                            if qs >= 6:
                                tensor.wait_ge(s_gc, qs - 5)
                            else:
                                tensor.wait_ge(s_a0, 96 - 6 + qs + 1)
                            pa = pg[slot(kf, g)][:, 0:CH]
                            m = g * 8 + k_out
                            for k in range(8):
                                if kf == 0 and g == 0:
                                    # sweep-(j-1) scans for H chunk k done
                                    tensor.wait_ge(s_hs, 2 + 32 * (j - 1) + 4 * (k + 1))
                                mm = tensor.matmul(
                                    pa, wv[:, k, m, :],
                                    hbuf[:, k * BT + st:k * BT + st + CH],
                                    start=(k == 0), stop=(k == 7))
                            mm.then_inc(s_pa, 1)
                # beta pre-act from final H
                hf = hb[NSWEEP % 2]
                for f in range(NF):
                    if f > 0:
                        tensor.wait_ge(s_ab, f)
                    st = hoff(f)
                    for k in range(8):
                        if f == 0:
                            tensor.wait_ge(s_hs, 2 + 32 * NSWEEP + 4 * (k + 1))
                        mm = tensor.matmul(psb[:, 0:CH], bw_s[:, k:k + 1],
                                           hf[:, k * BT + st:k * BT + st + CH],
                                           start=(k == 0), stop=(k == 7))
                    mm.then_inc(s_pb, 1)

            @block.scalar
            def _(scalar):
                scalar.wait_ge(dma_sem, 32)
                # phase 0: store biased xr/xz/xn tiles; sweep-0 gates from psum
                for kf in range(32):
                    k_out, f = kf // NF, kf % NF
                    fb = (kf % 2) * CH
                    for g in range(3):
                        i = kf * 3 + g
                        m = g * 8 + k_out
                        scalar.wait_ge(s_pa, i + 1)
                        if i >= XO:
                            scalar.wait_ge(s_xod[i % XO], 16 * (i // XO))
                        xo = xout_s[:, (i % XO) * CH:(i % XO + 1) * CH]
                        pa = pg[slot(kf, g)][:, 0:CH]
                        if g == 0:
                            scalar.activation(xo, pa, AF.Identity,
                                              bias=bias_s[:, m:m + 1]).then_inc(s_a0, 1)
                        elif g == 1:
                            scalar.activation(xo, pa, AF.Identity,
                                              bias=bias_s[:, m:m + 1]).then_inc(s_xo, 1)
                            if kf >= 2:
                                # gate-tile slot reuse: DVE readers of kf-2 done
                                scalar.wait_ge(s_nz, kf - 1)
                                scalar.wait_ge(s_hs, kf + 1)
                            scalar.activation(zb_s[:, fb:fb + CH], pa, AF.Sigmoid,
                                              bias=bias_s[:, m:m + 1])
                            scalar.activation(z_s[:, fb:fb + CH], pa, AF.Sigmoid,
                                              scale=-1.0,
                                              bias=bias_s[:, 24 + k_out:25 + k_out]
                                              ).then_inc(s_a0, 1)
                        else:
                            scalar.activation(xo, pa, AF.Identity,
                                              bias=bias_s[:, m:m + 1]).then_inc(s_xo, 1)
                            scalar.activation(n_s[:, fb:fb + CH], pa, AF.Tanh,
                                              bias=bias_s[:, m:m + 1]).then_inc(s_a0, 1)
                # sweeps
                for j in range(1, NSWEEP + 1):
                    for kf in range(32):
                        cnt = (j - 1) * 32 + kf + 1
                        gcb = (j - 1) * 96 + kf * 3
                        fb = (kf % 2) * CH
                        scalar.wait_ge(s_gc, gcb + 2)   # tr,tz written (DVE)
                        scalar.activation(r_s[:, fb:fb + CH], tr_s[:, fb:fb + CH],
                                          AF.Sigmoid).then_inc(s_a1, 1)
                        scalar.activation(zb_s[:, fb:fb + CH], tz_s[:, fb:fb + CH],
                                          AF.Sigmoid)
                        scalar.activation(z_s[:, fb:fb + CH], tz_s[:, fb:fb + CH],
                                          AF.Sigmoid, scale=-1.0)
                        scalar.wait_ge(s_d2, cnt)
                        scalar.activation(n_s[:, fb:fb + CH], tn2_s[:, fb:fb + CH],
                                          AF.Tanh).then_inc(s_a2, 1)
                for f in range(NF):
                    scalar.wait_ge(s_pb, f + 1)
                    scalar.activation(betap_s[:, fcols(f)], psb[:, 0:CH],
                                      AF.Copy).then_inc(s_ab, 1)

            @block.vector
            def _(vector):
                vector.memset(hb0[:], 0.0).then_inc(s_hs, 1)
                vector.memset(hb1[:], 0.0).then_inc(s_hs, 1)
                vector.wait_ge(dma_sem, 32)
                # sweep 0: nz + scan per (k_out, f) into hb0
                for kf in range(32):
                    k_out, f = kf // NF, kf % NF
                    fb = (kf % 2) * CH
                    vector.wait_ge(s_a0, 3 * (kf + 1))
                    vector.tensor_mul(nz_s[:, fb:fb + CH], zb_s[:, fb:fb + CH],
                                      n_s[:, fb:fb + CH]).then_inc(s_nz, 1)
                    vector.wait_ge(s_nz, kf + 1)
                    vector.wait_ge(s_hs, 2 + kf)
                    st = k_out * BT + hoff(f)
                    vector.tensor_tensor_scan(hb0[:, st:st + CH],
                                              z_s[:, fb:fb + CH],
                                              nz_s[:, fb:fb + CH],
                                              hb0[:, st - 1:st],
                                              mybir.AluOpType.mult,
                                              mybir.AluOpType.add).then_inc(s_hs, 1)
                # sweeps
                for j in range(1, NSWEEP + 1):
                    hw = hb[j % 2]
                    for kf in range(32):
                        k_out, f = kf // NF, kf % NF
                        cnt = (j - 1) * 32 + kf + 1
                        base = 96 + (j - 1) * 96 + kf * 3
                        ii = (j - 1) * 96 + kf * 3
                        fb = (kf % 2) * CH
                        vector.wait_ge(s_xin[ii % XR], 16 * (ii // XR + 1))
                        vector.wait_ge(s_pa, base + 1)
                        vector.tensor_add(tr_s[:, fb:fb + CH],
                                          pg[slot(kf, 0)][:, 0:CH],
                                          xin_s[:, (ii % XR) * CH:(ii % XR + 1) * CH]
                                          ).then_inc(s_gc, 1)
                        vector.wait_ge(s_xin[(ii + 1) % XR], 16 * ((ii + 1) // XR + 1))
                        vector.wait_ge(s_pa, base + 2)
                        vector.tensor_add(tz_s[:, fb:fb + CH],
                                          pg[slot(kf, 1)][:, 0:CH],
                                          xin_s[:, ((ii + 1) % XR) * CH:
                                                ((ii + 1) % XR + 1) * CH]
                                          ).then_inc(s_gc, 1)
                        vector.wait_ge(s_a1, cnt)
                        vector.wait_ge(s_pa, base + 3)
                        gcb = (j - 1) * 96 + kf * 3
                        vector.tensor_mul(tn_s[:, fb:fb + CH], r_s[:, fb:fb + CH],
                                          pg[slot(kf, 2)][:, 0:CH]).then_inc(s_gc, 1)
                        vector.wait_ge(s_gc, gcb + 3)
                        vector.wait_ge(s_xin[(ii + 2) % XR], 16 * ((ii + 2) // XR + 1))
                        vector.tensor_add(tn2_s[:, fb:fb + CH], tn_s[:, fb:fb + CH],
                                          xin_s[:, ((ii + 2) % XR) * CH:
                                                ((ii + 2) % XR + 1) * CH]
                                          ).then_inc(s_d2, 1)
                        vector.wait_ge(s_a2, cnt)
                        vector.tensor_mul(nz_s[:, fb:fb + CH], zb_s[:, fb:fb + CH],
                                          n_s[:, fb:fb + CH]).then_inc(s_nz, 1)
                        vector.wait_ge(s_nz, j * 32 + kf + 1)
                        vector.wait_ge(s_hs, 2 + j * 32 + kf)
                        st = k_out * BT + hoff(f)
                        vector.tensor_tensor_scan(hw[:, st:st + CH],
                                                  z_s[:, fb:fb + CH],
                                                  nz_s[:, fb:fb + CH],
                                                  hw[:, st - 1:st],
                                                  mybir.AluOpType.mult,
                                                  mybir.AluOpType.add).then_inc(s_hs, 1)
    return nc


def p1r_finish(results):
    pre = np.asarray(results[0]["betap"], np.float64).reshape(B, N)
    return (1.0 / (1.0 + np.exp(-pre))).astype(np.float32)


# ------------------------------------------------------------ P2 (scan+dec)
def _p2_host_prep(inputs, core):
    lat = np.asarray(inputs["latent"], np.float32)
    dec_w1 = np.asarray(inputs["dec_w1"], np.float32)
    dec_b1 = np.asarray(inputs["dec_b1"], np.float32)
    dec_w2 = np.asarray(inputs["dec_w2"], np.float32)
    dec_b2 = np.asarray(inputs["dec_b2"], np.float32)
    c = core
    bf = ml_dtypes.bfloat16

    d_perm = np.concatenate([np.arange(c * P, (c + 1) * P),
                             np.delete(np.arange(D), np.arange(c * P, (c + 1) * P))])
    latTd = np.ascontiguousarray(lat.transpose(2, 0, 1).reshape(D, B * N)[d_perm], np.float32)
    rows = (c * P + np.arange(P)[None, :]) * R + np.arange(R)[:, None]
    w2T_shard = np.ascontiguousarray(dec_w2[rows.reshape(-1), :].T).astype(bf)
    b2w1 = np.ascontiguousarray(dec_b2[rows], np.float32)
    W2s = dec_w2[D * R:].reshape(D, R, H).sum(0)
    b2s = dec_b2[D * R:].reshape(D, R).sum(0)[:, None]
    return {
        "latTd": latTd,
        "w1T": np.ascontiguousarray(dec_w1[:, d_perm].T).astype(bf),
        "b1_pc": np.ascontiguousarray(dec_b1.reshape(16, P).T, np.float32),
        "W2sT": np.ascontiguousarray(W2s.T).astype(bf),
        "b2s_pc": np.ascontiguousarray(b2s, np.float32),
        "w2T_shard": w2T_shard,
        "b2w1": b2w1,
    }


def _p2_beta_prep(beta):
    return {"bbc": np.ascontiguousarray(
        np.repeat(beta.reshape(1, B * N), P, axis=0), np.float32)}


def _p2_build(nc):
    from contextlib import ExitStack
    latTd = nc.declare_dram_parameter("latTd", [D, B * N], F32, isOutput=False)
    bbc = nc.declare_dram_parameter("bbc", [P, B * N], F32, isOutput=False)
    w1T = nc.declare_dram_parameter("w1T", [D, H], BF16, isOutput=False)
    b1_pc = nc.declare_dram_parameter("b1_pc", [P, 16], F32, isOutput=False)
    W2sT = nc.declare_dram_parameter("W2sT", [H, R], BF16, isOutput=False)
    b2s_pc = nc.declare_dram_parameter("b2s_pc", [R, 1], F32, isOutput=False)
    w2T_shard = nc.declare_dram_parameter("w2T_shard", [H, H], BF16, isOutput=False)
    b2w1 = nc.declare_dram_parameter("b2w1", [R, P], F32, isOutput=False)
    outT = nc.declare_dram_parameter("outT", [P, B * N], F32, isOutput=True)
    w2s_dram = nc.dram_tensor("w2s_dram", [R, B * N], F32)

    with TileContext(nc) as tc, ExitStack() as ctx:
        const = ctx.enter_context(tc.tile_pool(name="const", bufs=1))
        persist = ctx.enter_context(tc.tile_pool(name="persist", bufs=1))
        lhs_pool = ctx.enter_context(tc.tile_pool(name="lhs", bufs=4))
        work = ctx.enter_context(tc.tile_pool(name="work", bufs=3))
        pbig = ctx.enter_context(tc.tile_pool(name="pbig", bufs=2, space="PSUM"))
        psmall = ctx.enter_context(tc.tile_pool(name="psmall", bufs=2, space="PSUM"))

        b1t = const.tile([P, 16], F32, tag="b1t")
        nc.sync.dma_start(out=b1t[:], in_=b1_pc[:, :])
        b2st = const.tile([R, 1], F32, tag="b2st")
        nc.sync.dma_start(out=b2st[:], in_=b2s_pc[:, :])
        b2w1t = const.tile([R, P], F32, tag="b2w1t")
        nc.sync.dma_start(out=b2w1t[:], in_=b2w1[:, :])
        latTt = const.tile([P, B * N], F32, tag="latTt")
        nc.sync.dma_start(out=latTt[:], in_=latTd[0:P, :])
        bbct = const.tile([P, B * N], F32, tag="bbct")
        nc.sync.dma_start(out=bbct[:], in_=bbc[:, :])

        gT = [[persist.tile([P, N], BF16, tag=f"g{b}_{dm}", name=f"g{b}_{dm}") for dm in range(8)]
              for b in range(B)]
        gown = persist.tile([P, B * N], F32, tag="gown")
        hid = [persist.tile([P, B * N], BF16, tag=f"hid{m}", name=f"hid{m}") for m in range(16)]
        w2st = persist.tile([R, B * N], F32, tag="w2st")
        acc = persist.tile([P, B * N], F32, tag="acc")

        # Phase 1: gated scan
        for dm in range(8):
            ldt = work.tile([P, B * N], F32, tag="ldt", bufs=1, name="ldt")
            nc.sync.dma_start(out=ldt[:], in_=latTd[dm * P:(dm + 1) * P, :])
            for b in range(B):
                sl = slice(b * N, (b + 1) * N)
                if dm == 0:
                    nc.vector.tensor_tensor_scan(gown[:, sl], bbct[:, sl], ldt[:, sl],
                                                 0.0, mybir.AluOpType.mult,
                                                 mybir.AluOpType.add)
                    nc.scalar.activation(gT[b][0][:, :], gown[:, sl], AF.Copy)
                else:
                    nc.vector.tensor_tensor_scan(gT[b][dm][:, :], bbct[:, sl], ldt[:, sl],
                                                 0.0, mybir.AluOpType.mult,
                                                 mybir.AluOpType.add)

        # Phase 2: mm1 -> hid (gelu tanh-approx == x*sigmoid(1.5957691216*(x+0.044715x^3)))
        w2h = persist.tile([P, 8 * H], BF16, tag="w2h")
        for k in range(8):
            nc.sync.dma_start(out=w2h[:, k * H:(k + 1) * H],
                              in_=w1T[k * P:(k + 1) * P, :])
        for m in range(16):
            for b in range(B):
                ph = pbig.tile([P, N], F32, tag="big", name="ph")
                for k in range(8):
                    for jj in range(2):
                        nc.tensor.matmul(ph[:, jj * 512:(jj + 1) * 512],
                                         w2h[:, k * H + m * P:k * H + (m + 1) * P],
                                         gT[b][k][:, jj * 512:(jj + 1) * 512],
                                         start=(k == 0), stop=(k == 7))
                xg = work.tile([P, N], F32, tag="xg", bufs=2, name="xg")
                nc.scalar.activation(xg[:], ph[:], AF.Identity, bias=b1t[:, m:m + 1])
                ta = work.tile([P, N], F32, tag="tmpA", bufs=2, name="ta")
                nc.scalar.activation(ta[:], xg[:], AF.Square, scale=0.21146040470)
                tb = work.tile([P, N], F32, tag="tmpB", bufs=2, name="tb")
                nc.vector.tensor_mul(tb[:], ta[:], xg[:])
                ta2 = work.tile([P, N], F32, tag="tmpA", bufs=2, name="ta2")
                nc.vector.tensor_add(ta2[:], xg[:], tb[:])
                tb2 = work.tile([P, N], F32, tag="tmpB", bufs=2, name="tb2")
                nc.scalar.activation(tb2[:], ta2[:], AF.Sigmoid, scale=1.5957691216)
                nc.vector.tensor_mul(hid[m][:, b * N:(b + 1) * N], xg[:], tb2[:])

        # Phase 3: w2s
        for n in range(2):
            pw = pbig.tile([R, N], F32, tag="big", name="pw")
            for k in range(16):
                wt = lhs_pool.tile([P, R], BF16, tag="w2slhs", name="w2slhs")
                nc.sync.dma_start(out=wt[:], in_=W2sT[k * P:(k + 1) * P, :])
                for jj in range(2):
                    nc.tensor.matmul(pw[:, jj * 512:(jj + 1) * 512], wt[:],
                                     hid[k][:, n * N + jj * 512:n * N + (jj + 1) * 512],
                                     start=(k == 0), stop=(k == 15))
            nc.scalar.activation(w2st[:, n * N:(n + 1) * N], pw[:], AF.Identity,
                                 bias=b2st[:, 0:1])
            nc.sync.dma_start(out=w2s_dram[:, n * N:(n + 1) * N], in_=w2st[:, n * N:(n + 1) * N])

        # Phase 4: acc seed + mm2 + r-contraction
        for n in range(4):
            psd = psmall.tile([P, 512], F32, tag="small", name="psd")
            nc.tensor.matmul(psd[:], b2w1t[:], w2st[:, n * 512:(n + 1) * 512],
                             start=True, stop=True)
            nc.scalar.activation(acc[:, n * 512:(n + 1) * 512], psd[:], AF.Copy)

        for h in range(2):
            for kk in range(8):
                k = h * 8 + kk
                nc.sync.dma_start(out=w2h[:, kk * H:(kk + 1) * H],
                                  in_=w2T_shard[k * P:(k + 1) * P, :])
            for m in range(16):
                for n in range(2):
                    pm = pbig.tile([P, N], F32, tag="big", name="pm")
                    for kk in range(8):
                        k = h * 8 + kk
                        for jj in range(2):
                            nc.tensor.matmul(pm[:, jj * 512:(jj + 1) * 512],
                                             w2h[:, kk * H + m * P:kk * H + (m + 1) * P],
                                             hid[k][:, n * N + jj * 512:n * N + (jj + 1) * 512],
                                             start=(kk == 0), stop=(kk == 7))
                    wb = work.tile([P, N], F32, tag="tmpA", bufs=2, name="wb")
                    nc.sync.dma_start(out=wb[:], in_=w2s_dram[m:m + 1, n * N:(n + 1) * N]
                                      .to_broadcast([P, N]))
                    tmp = work.tile([P, N], F32, tag="tmpB", bufs=2, name="tmp")
                    nc.vector.tensor_mul(tmp[:], pm[:], wb[:])
                    nc.vector.tensor_add(acc[:, n * N:(n + 1) * N],
                                         acc[:, n * N:(n + 1) * N], tmp[:])

        # Phase 5: out = latT + gown * acc
        for n in range(2):
            sl = slice(n * N, (n + 1) * N)
            ctrl = work.tile([P, N], F32, tag="tmpA", bufs=2, name="ctrl")
            nc.vector.tensor_mul(ctrl[:], acc[:, sl], gown[:, sl])
            ot = work.tile([P, N], F32, tag="tmpB", bufs=2, name="ot")
            nc.vector.tensor_add(ot[:], ctrl[:], latTt[:, sl])
            nc.sync.dma_start(out=outT[:, sl], in_=ot[:])
    return nc


def _p2_finish(results):
    out = np.empty((B, N, D), np.float32)
    for c in range(8):
        o = np.asarray(results[c]["outT"])
        out[:, :, c * P:(c + 1) * P] = o.reshape(P, B, N).transpose(1, 2, 0)
    return out


# ----------------------------------------------------------------- kernel()
_cache = {}


def _get_programs():
    if "nc1" not in _cache:
        nc1 = bass.Bass()
        p1r_build(nc1)
        _cache["nc1"] = nc1
        nc2 = bacc.Bacc(None, target_bir_lowering=False)
        _p2_build(nc2)
        nc2.finalize()
        _cache["nc2"] = nc2
    return _cache["nc1"], _cache["nc2"]


def _get_maps(inputs):
    key = tuple(id(inputs[k]) for k in sorted(inputs))
    if _cache.get("maps_key") != key:
        _cache["maps1"] = [p1r_host_prep(inputs)] * 8
        _cache["maps2"] = [_p2_host_prep(inputs, c) for c in range(8)]
        _cache["maps_key"] = key
    return _cache["maps1"], _cache["maps2"]


LAST_EXEC_NS = [None, None]


def kernel(**inputs):
    nc1, nc2 = _get_programs()
    maps1, maps2s = _get_maps(inputs)
    r1 = run_bass_kernel_spmd(nc1, maps1, list(range(8)))
    beta = p1r_finish(r1.results)
    bmap = _p2_beta_prep(beta)
    maps2 = [dict(m, **bmap) for m in maps2s]
    r2 = run_bass_kernel_spmd(nc2, maps2, list(range(8)))
    LAST_EXEC_NS[0] = r1.exec_time_ns
    LAST_EXEC_NS[1] = r2.exec_time_ns
    return _p2_finish(r2.results)


# revision 28
# speedup vs baseline: 1.0119x; 1.0119x over previous
"""Trainium2 Bass kernel for nn_MetaController (GRU + gated scan + hypernet decoder).

Self-contained: kernel(**inputs) -> np.ndarray [2,1024,1024] float32.

Two SPMD programs on 8 NeuronCores:
  P1: GRU solved by fixed-point iteration (scan-variant): 7 sweeps total
      (sweep 0 elementwise, 6 sweeps with the h->3h matmul batched over all
      2048 tokens). Each sweep: hp = H_prev_shifted @ w_hh^T (8-way
      channel-sharded, bf16), gates on ACT/DVE, z-diagonal recurrence solved
      exactly via DVE tensor_tensor_scan, then the 128-channel H chunk is
      broadcast to all cores via remote SBUF DMA. Converges to the bf16
      floor (~1e-4 end-to-end) in 6 matmul sweeps. Emits per-core partial
      beta projections; host sums + applies sigmoid.
  P2: gated associative scan via DVE tensor_tensor_scan, decoder mm1 (gelu)
      replicated, 16384-row w1-half of the decoder output tensor-parallel in
      r-major row order so the low-rank contraction sum_r w1*(w2 row-sums)
      becomes 16 broadcast-multiply-accumulates. The w2-half collapses to 16
      columns via host-presummed W2s.
"""
import sys
sys.path.insert(0, '/opt/trn_rl_repo')
import numpy as np
import ml_dtypes
import concourse.bass as bass
import concourse.mybir as mybir
from concourse.bass import ds
from concourse import library_config, library_overlay, bacc
from concourse.tile import TileContext
from concourse.bass_utils import run_bass_kernel_spmd

F32 = mybir.dt.float32
BF16 = mybir.dt.bfloat16
I32 = mybir.dt.int32
AF = mybir.ActivationFunctionType

B, N, D, R, H = 2, 1024, 1024, 16, 2048
P = 128
T = B * N            # 2048 tokens, order b*N + n
TPAD = N + 1         # per-chain padded length (zero col at chain start)
BT = B * TPAD        # 2050
NSWEEP = 2           # matmul sweeps (plus sweep 0 without matmul)
CH = 512             # token f-chunk
NF = T // CH         # 4 f-chunks (2 per chain)
NM = 24              # P1 m-tiles: m = g*8 + k_out
XR = 16              # P1 xin ring slots
XO = 8               # P1 xout ring slots


# ------------------------------------------------------------------ P1 (GRU)
def p1r_host_prep(inputs):
    w_ih = np.asarray(inputs["gru_w_ih"], np.float32)
    w_hh = np.asarray(inputs["gru_w_hh"], np.float32)
    b_ih = np.asarray(inputs["gru_b_ih"], np.float32)
    b_hh = np.asarray(inputs["gru_b_hh"], np.float32)
    beta_w = np.asarray(inputs["beta_w"], np.float32)
    lat = np.asarray(inputs["latent"], np.float32)
    bf = ml_dtypes.bfloat16

    latT = np.ascontiguousarray(lat.transpose(2, 0, 1).reshape(D, T))
    sgn = np.array([1.0, -1.0, 1.0], np.float32)
    wih = np.concatenate([sgn[g] * w_ih[g * D:(g + 1) * D] for g in range(3)], 0)
    whh = np.concatenate([sgn[g] * w_hh[g * D:(g + 1) * D] for g in range(3)], 0)
    assert not np.any(b_hh[2 * D:]), "b_hh n-gate must be zero"
    bias = np.concatenate([(b_ih[:2 * D] + b_hh[:2 * D]) * np.repeat(sgn[:2], D),
                           b_ih[2 * D:]])  # [3D]; m-block col = bias[m*128:...]
    bias24 = bias.reshape(NM, P).T                    # [P, 24]
    biasz_neg = -bias24[:, 8:16]                      # [P, 8] (z chunks, negated)
    bias_pc = np.concatenate([bias24, biasz_neg], 1)  # [P, 32]
    return {
        "latT_tb": latT.astype(bf),
        "wih_l": np.ascontiguousarray(wih.T).astype(bf),   # [D, 3D] lhsT
        "whh_l": np.ascontiguousarray(whh.T).astype(bf),
        "bias_pc": np.ascontiguousarray(bias_pc, np.float32),
        "bw_pc": np.ascontiguousarray(beta_w.reshape(8, P).T).astype(bf),  # [P,8]
    }


def p1r_build(nc):
    latT_tb = nc.declare_dram_parameter("latT_tb", [D, T], BF16, isOutput=False)
    wih_l = nc.declare_dram_parameter("wih_l", [D, 3 * D], BF16, isOutput=False)
    whh_l = nc.declare_dram_parameter("whh_l", [D, 3 * D], BF16, isOutput=False)
    bias_pc = nc.declare_dram_parameter("bias_pc", [P, 32], F32, isOutput=False)
    bw_pc = nc.declare_dram_parameter("bw_pc", [P, 8], BF16, isOutput=False)
    betap = nc.declare_dram_parameter("betap", [1, T], F32, isOutput=True)
    xp_d = nc.dram_tensor("xp_d", [NM * P, T], BF16)

    from contextlib import ExitStack
    with ExitStack() as ctx:
        def sbuf(name, shape, dtype):
            return ctx.enter_context(nc.sbuf_tensor(name, shape, dtype))

        def sem(name):
            return ctx.enter_context(nc.semaphore(name))

        latT_s = sbuf("latT_s", [P, 8 * T], BF16)
        w_s = sbuf("w_s", [P, 192 * P], BF16)       # wih for phase 0, whh after
        hb0 = sbuf("hb0", [P, 8 * BT], BF16)
        hb1 = sbuf("hb1", [P, 8 * BT], BF16)
        xin_s = sbuf("xin_s", [P, XR * CH], BF16)
        xout_s = sbuf("xout_s", [P, XO * CH], BF16)
        bias_s = sbuf("bias_s", [P, 32], F32)
        bw_s = sbuf("bw_s", [P, 8], BF16)
        tr_s = sbuf("tr_s", [P, 2 * CH], F32)
        tz_s = sbuf("tz_s", [P, 2 * CH], F32)
        tn_s = sbuf("tn_s", [P, 2 * CH], F32)
        tn2_s = sbuf("tn2_s", [P, 2 * CH], F32)
        r_s = sbuf("r_s", [P, 2 * CH], BF16)
        zb_s = sbuf("zb_s", [P, 2 * CH], BF16)
        z_s = sbuf("z_s", [P, 2 * CH], BF16)
        n_s = sbuf("n_s", [P, 2 * CH], BF16)
        nz_s = sbuf("nz_s", [P, 2 * CH], BF16)
        betap_s = sbuf("betap_s", [1, T], F32)

        pg = [ctx.enter_context(nc.psum_tensor(f"pg{i}", [P, CH], F32))
              for i in range(6)]
        psb = ctx.enter_context(nc.psum_tensor("psb", [1, CH], F32))

        dma_sem = sem("dma_sem")
        s_pa = sem("s_pa")    # PE psum group done (cumulative)
        s_a0 = sem("s_a0")    # phase-0 psum fully consumed (ACT, 1/group)
        s_gc = sem("s_gc")    # sweep psum+xin consumed (DVE tr/tz/tn2)
        s_a1 = sem("s_a1")    # ACT r ready (sweeps, 1/(k,f))
        s_tn = sem("s_tn")    # DVE tn retired (self-sync)
        s_d2 = sem("s_d2")    # DVE tn2 ready
        s_a2 = sem("s_a2")    # ACT zb,z,n done (sweeps, 1/(k,f))
        s_nz = sem("s_nz")    # DVE nz retired
        s_hs = sem("s_hs")    # scan chunk done (memsets count 2)
        s_xo = sem("s_xo")    # ACT wrote xout slot
        s_xod = [sem(f"s_xod{s}") for s in range(XO)]  # per-slot xout->DRAM done
        s_xin = [sem(f"s_xin{s}") for s in range(XR)]  # per-slot xin landed
        s_whk = [sem(f"s_whk{k}") for k in range(8)]  # whh k_out block loaded
        s_pb = sem("s_pb")
        s_ab = sem("s_ab")

        lat3 = latT_s[:].rearrange("p (k c) -> p k c", k=8)
        wv = w_s[:].rearrange("p (k m c) -> p k m c", k=8, m=NM)
        hb = [hb0, hb1]

        def fcols(f):
            return slice(f * CH, (f + 1) * CH)

        def slot(kf, g):
            return (kf % 2) * 3 + g

        def hoff(f):  # data col offset of chunk f inside one padded 2050 block
            return (f // 2) * TPAD + 1 + (f % 2) * CH

        with nc.Block() as block:
            @block.sync
            def _(sync):
                sync.dma_start(out=lat3, in_=latT_tb[:, :]
                               .rearrange("(k p) c -> p k c", p=P)).then_inc(dma_sem, 16)
                sync.dma_start(out=wv, in_=wih_l[:, :]
                               .rearrange("(k p) (m c) -> p k m c", p=P, m=NM)
                               ).then_inc(dma_sem, 16)
                sync.dma_start(out=bias_s[:], in_=bias_pc[:, :]).then_inc(dma_sem, 16)
                sync.dma_start(out=bw_s[:], in_=bw_pc[:, :]).then_inc(dma_sem, 16)
                # xout -> DRAM as ACT produces tiles (96)
                xoc = 0
                for i in range(NM * NF):
                    kf, g = i // 3, i % 3
                    k_out, f = kf // NF, kf % NF
                    m = g * 8 + k_out
                    if g == 0:
                        sync.wait_ge(s_a0, i + 1)
                    else:
                        xoc += 1
                        sync.wait_ge(s_xo, xoc)
                    sync.dma_start(out=xp_d[m * P:(m + 1) * P, fcols(f)],
                                   in_=xout_s[:, (i % XO) * CH:(i % XO + 1) * CH]
                                   ).then_inc(s_xod[i % XO], 16)
                # reload w_s with whh per (g, k_out) block as phase-0 frees it
                for ko in range(8):
                    sync.wait_ge(s_pa, 12 * ko + 12)
                    for g in range(3):
                        m = g * 8 + ko
                        sync.dma_start(out=wv[:, :, m, :],
                                       in_=whh_l[:, m * P:(m + 1) * P]
                                       .rearrange("(k p) c -> p k c", p=P)
                                       ).then_inc(s_whk[ko], 16)
                # xin streaming for sweeps (tile index i_in = (j-1)*96+kf*3+t)
                for j in range(1, NSWEEP + 1):
                    for kf in range(32):
                        k_out, f = kf // NF, kf % NF
                        for t in range(3):          # 0=xr 1=xz 2=xn
                            i_in = (j - 1) * 96 + kf * 3 + t
                            m = t * 8 + k_out
                            i_out = kf * 3 + t
                            sync.wait_ge(s_xod[i_out % XO], 16 * (i_out // XO + 1))
                            if i_in >= XR:
                                # slot last held tile i_in-XR; wait its consumer
                                ip = i_in - XR
                                jp = ip // 96
                                kfp = (ip % 96) // 3
                                tp = ip % 3
                                if tp < 2:
                                    sync.wait_ge(s_gc, jp * 96 + kfp * 3 + tp + 1)
                                else:
                                    sync.wait_ge(s_d2, jp * 32 + kfp + 1)
                            sync.dma_start(out=xin_s[:, (i_in % XR) * CH:
                                                     (i_in % XR + 1) * CH],
                                           in_=xp_d[m * P:(m + 1) * P, fcols(f)]
                                           ).then_inc(s_xin[i_in % XR], 16)
                sync.wait_ge(s_ab, NF)
                sync.dma_start(out=betap[:, :], in_=betap_s[:]).then_inc(dma_sem, 16)
                sync.wait_ge(dma_sem, 80)

            @block.tensor
            def _(tensor):
                tensor.wait_ge(dma_sem, 64)
                # phase 0: xp (= sweep-0 pre-acts) from latent
                for kf in range(32):
                    k_out, f = kf // NF, kf % NF
                    for g in range(3):
                        q = kf * 3 + g
                        if q >= 6:
                            tensor.wait_ge(s_a0, q - 5)
                        pa = pg[slot(kf, g)][:, 0:CH]
                        m = g * 8 + k_out
                        for k in range(8):
                            mm = tensor.matmul(pa, wv[:, k, m, :],
                                               lat3[:, k, fcols(f)],
                                               start=(k == 0), stop=(k == 7))
                        mm.then_inc(s_pa, 1)
                # sweeps
                for j in range(1, NSWEEP + 1):
                    hbuf = hb[(j - 1) % 2]
                    for kf in range(32):
                        k_out, f = kf // NF, kf % NF
                        if j == 1 and kf % NF == 0:
                            tensor.wait_ge(s_whk[k_out], 48)
                        chain, half = f // 2, f % 2
                        st = chain * TPAD + half * CH  # shifted (t-1) cols
                        for g in range(3):
                            qs = (j - 1) * 96 + kf * 3 + g
                            if qs >= 6:
                                tensor.wait_ge(s_gc, qs - 5)
                            else:
                                tensor.wait_ge(s_a0, 96 - 6 + qs + 1)
                            pa = pg[slot(kf, g)][:, 0:CH]
                            m = g * 8 + k_out
                            for k in range(8):
                                if kf == 0 and g == 0:
                                    # sweep-(j-1) scans for H chunk k done
                                    tensor.wait_ge(s_hs, 2 + 32 * (j - 1) + 4 * (k + 1))
                                mm = tensor.matmul(
                                    pa, wv[:, k, m, :],
                                    hbuf[:, k * BT + st:k * BT + st + CH],
                                    start=(k == 0), stop=(k == 7))
                            mm.then_inc(s_pa, 1)
                # beta pre-act from final H
                hf = hb[NSWEEP % 2]
                for f in range(NF):
                    if f > 0:
                        tensor.wait_ge(s_ab, f)
                    st = hoff(f)
                    for k in range(8):
                        if f == 0:
                            tensor.wait_ge(s_hs, 2 + 32 * NSWEEP + 4 * (k + 1))
                        mm = tensor.matmul(psb[:, 0:CH], bw_s[:, k:k + 1],
                                           hf[:, k * BT + st:k * BT + st + CH],
                                           start=(k == 0), stop=(k == 7))
                    mm.then_inc(s_pb, 1)

            @block.scalar
            def _(scalar):
                scalar.wait_ge(dma_sem, 64)
                # phase 0: store biased xr/xz/xn tiles; sweep-0 gates from psum
                for kf in range(32):
                    k_out, f = kf // NF, kf % NF
                    fb = (kf % 2) * CH
                    for g in range(3):
                        i = kf * 3 + g
                        m = g * 8 + k_out
                        scalar.wait_ge(s_pa, i + 1)
                        if i >= XO:
                            scalar.wait_ge(s_xod[i % XO], 16 * (i // XO))
                        xo = xout_s[:, (i % XO) * CH:(i % XO + 1) * CH]
                        pa = pg[slot(kf, g)][:, 0:CH]
                        if g == 0:
                            scalar.activation(xo, pa, AF.Identity,
                                              bias=bias_s[:, m:m + 1]).then_inc(s_a0, 1)
                        elif g == 1:
                            scalar.activation(xo, pa, AF.Identity,
                                              bias=bias_s[:, m:m + 1]).then_inc(s_xo, 1)
                            if kf >= 2:
                                # gate-tile slot reuse: DVE readers of kf-2 done
                                scalar.wait_ge(s_nz, kf - 1)
                                scalar.wait_ge(s_hs, kf + 1)
                            scalar.activation(zb_s[:, fb:fb + CH], pa, AF.Sigmoid,
                                              bias=bias_s[:, m:m + 1])
                            scalar.activation(z_s[:, fb:fb + CH], pa, AF.Sigmoid,
                                              scale=-1.0,
                                              bias=bias_s[:, 24 + k_out:25 + k_out]
                                              ).then_inc(s_a0, 1)
                        else:
                            scalar.activation(xo, pa, AF.Identity,
                                              bias=bias_s[:, m:m + 1]).then_inc(s_xo, 1)
                            scalar.activation(n_s[:, fb:fb + CH], pa, AF.Tanh,
                                              bias=bias_s[:, m:m + 1]).then_inc(s_a0, 1)
                # sweeps
                for j in range(1, NSWEEP + 1):
                    for kf in range(32):
                        cnt = (j - 1) * 32 + kf + 1
                        gcb = (j - 1) * 96 + kf * 3
                        fb = (kf % 2) * CH
                        scalar.wait_ge(s_gc, gcb + 2)   # tr,tz written (DVE)
                        scalar.activation(r_s[:, fb:fb + CH], tr_s[:, fb:fb + CH],
                                          AF.Sigmoid).then_inc(s_a1, 1)
                        scalar.activation(zb_s[:, fb:fb + CH], tz_s[:, fb:fb + CH],
                                          AF.Sigmoid)
                        scalar.activation(z_s[:, fb:fb + CH], tz_s[:, fb:fb + CH],
                                          AF.Sigmoid, scale=-1.0)
                        scalar.wait_ge(s_d2, cnt)
                        scalar.activation(n_s[:, fb:fb + CH], tn2_s[:, fb:fb + CH],
                                          AF.Tanh).then_inc(s_a2, 1)
                for f in range(NF):
                    scalar.wait_ge(s_pb, f + 1)
                    scalar.activation(betap_s[:, fcols(f)], psb[:, 0:CH],
                                      AF.Copy).then_inc(s_ab, 1)

            @block.vector
            def _(vector):
                vector.memset(hb0[:], 0.0).then_inc(s_hs, 1)
                vector.memset(hb1[:], 0.0).then_inc(s_hs, 1)
                vector.wait_ge(dma_sem, 64)
                # sweep 0: nz + scan per (k_out, f) into hb0
                for kf in range(32):
                    k_out, f = kf // NF, kf % NF
                    fb = (kf % 2) * CH
                    vector.wait_ge(s_a0, 3 * (kf + 1))
                    vector.tensor_mul(nz_s[:, fb:fb + CH], zb_s[:, fb:fb + CH],
                                      n_s[:, fb:fb + CH]).then_inc(s_nz, 1)
                    vector.wait_ge(s_nz, kf + 1)
                    vector.wait_ge(s_hs, 2 + kf)
                    st = k_out * BT + hoff(f)
                    vector.tensor_tensor_scan(hb0[:, st:st + CH],
                                              z_s[:, fb:fb + CH],
                                              nz_s[:, fb:fb + CH],
                                              hb0[:, st - 1:st],
                                              mybir.AluOpType.mult,
                                              mybir.AluOpType.add).then_inc(s_hs, 1)
                # sweeps
                for j in range(1, NSWEEP + 1):
                    hw = hb[j % 2]
                    for kf in range(32):
                        k_out, f = kf // NF, kf % NF
                        cnt = (j - 1) * 32 + kf + 1
                        base = 96 + (j - 1) * 96 + kf * 3
                        ii = (j - 1) * 96 + kf * 3
                        fb = (kf % 2) * CH
                        vector.wait_ge(s_xin[ii % XR], 16 * (ii // XR + 1))
                        vector.wait_ge(s_pa, base + 1)
                        vector.tensor_add(tr_s[:, fb:fb + CH],
                                          pg[slot(kf, 0)][:, 0:CH],
                                          xin_s[:, (ii % XR) * CH:(ii % XR + 1) * CH]
                                          ).then_inc(s_gc, 1)
                        vector.wait_ge(s_xin[(ii + 1) % XR], 16 * ((ii + 1) // XR + 1))
                        vector.wait_ge(s_pa, base + 2)
                        vector.tensor_add(tz_s[:, fb:fb + CH],
                                          pg[slot(kf, 1)][:, 0:CH],
                                          xin_s[:, ((ii + 1) % XR) * CH:
                                                ((ii + 1) % XR + 1) * CH]
                                          ).then_inc(s_gc, 1)
                        vector.wait_ge(s_a1, cnt)
                        vector.wait_ge(s_pa, base + 3)
                        gcb = (j - 1) * 96 + kf * 3
                        vector.tensor_mul(tn_s[:, fb:fb + CH], r_s[:, fb:fb + CH],
                                          pg[slot(kf, 2)][:, 0:CH]).then_inc(s_gc, 1)
                        vector.wait_ge(s_gc, gcb + 3)
                        vector.wait_ge(s_xin[(ii + 2) % XR], 16 * ((ii + 2) // XR + 1))
                        vector.tensor_add(tn2_s[:, fb:fb + CH], tn_s[:, fb:fb + CH],
                                          xin_s[:, ((ii + 2) % XR) * CH:
                                                ((ii + 2) % XR + 1) * CH]
                                          ).then_inc(s_d2, 1)
                        vector.wait_ge(s_a2, cnt)
                        vector.tensor_mul(nz_s[:, fb:fb + CH], zb_s[:, fb:fb + CH],
                                          n_s[:, fb:fb + CH]).then_inc(s_nz, 1)
                        vector.wait_ge(s_nz, j * 32 + kf + 1)
                        vector.wait_ge(s_hs, 2 + j * 32 + kf)
                        st = k_out * BT + hoff(f)
                        vector.tensor_tensor_scan(hw[:, st:st + CH],
                                                  z_s[:, fb:fb + CH],
                                                  nz_s[:, fb:fb + CH],
                                                  hw[:, st - 1:st],
                                                  mybir.AluOpType.mult,
                                                  mybir.AluOpType.add).then_inc(s_hs, 1)
    return nc


def p1r_finish(results):
    pre = np.asarray(results[0]["betap"], np.float64).reshape(B, N)
    return (1.0 / (1.0 + np.exp(-pre))).astype(np.float32)


# ------------------------------------------------------------ P2 (scan+dec)
def _p2_host_prep(inputs, core):
    lat = np.asarray(inputs["latent"], np.float32)
    dec_w1 = np.asarray(inputs["dec_w1"], np.float32)
    dec_b1 = np.asarray(inputs["dec_b1"], np.float32)
    dec_w2 = np.asarray(inputs["dec_w2"], np.float32)
    dec_b2 = np.asarray(inputs["dec_b2"], np.float32)
    c = core
    bf = ml_dtypes.bfloat16

    d_perm = np.concatenate([np.arange(c * P, (c + 1) * P),
                             np.delete(np.arange(D), np.arange(c * P, (c + 1) * P))])
    latTd = np.ascontiguousarray(lat.transpose(2, 0, 1).reshape(D, B * N)[d_perm], np.float32)
    rows = (c * P + np.arange(P)[None, :]) * R + np.arange(R)[:, None]
    w2T_shard = np.ascontiguousarray(dec_w2[rows.reshape(-1), :].T).astype(bf)
    b2w1 = np.ascontiguousarray(dec_b2[rows], np.float32)
    W2s = dec_w2[D * R:].reshape(D, R, H).sum(0)
    b2s = dec_b2[D * R:].reshape(D, R).sum(0)[:, None]
    return {
        "latTd": latTd,
        "w1T": np.ascontiguousarray(dec_w1[:, d_perm].T).astype(bf),
        "b1_pc": np.ascontiguousarray(dec_b1.reshape(16, P).T, np.float32),
        "W2sT": np.ascontiguousarray(W2s.T).astype(bf),
        "b2s_pc": np.ascontiguousarray(b2s, np.float32),
        "w2T_shard": w2T_shard,
        "b2w1": b2w1,
    }


def _p2_beta_prep(beta):
    return {"bbc": np.ascontiguousarray(
        np.repeat(beta.reshape(1, B * N), P, axis=0), np.float32)}


def _p2_build(nc):
    from contextlib import ExitStack
    latTd = nc.declare_dram_parameter("latTd", [D, B * N], F32, isOutput=False)
    bbc = nc.declare_dram_parameter("bbc", [P, B * N], F32, isOutput=False)
    w1T = nc.declare_dram_parameter("w1T", [D, H], BF16, isOutput=False)
    b1_pc = nc.declare_dram_parameter("b1_pc", [P, 16], F32, isOutput=False)
    W2sT = nc.declare_dram_parameter("W2sT", [H, R], BF16, isOutput=False)
    b2s_pc = nc.declare_dram_parameter("b2s_pc", [R, 1], F32, isOutput=False)
    w2T_shard = nc.declare_dram_parameter("w2T_shard", [H, H], BF16, isOutput=False)
    b2w1 = nc.declare_dram_parameter("b2w1", [R, P], F32, isOutput=False)
    outT = nc.declare_dram_parameter("outT", [P, B * N], F32, isOutput=True)
    w2s_dram = nc.dram_tensor("w2s_dram", [R, B * N], F32)

    with TileContext(nc) as tc, ExitStack() as ctx:
        const = ctx.enter_context(tc.tile_pool(name="const", bufs=1))
        persist = ctx.enter_context(tc.tile_pool(name="persist", bufs=1))
        lhs_pool = ctx.enter_context(tc.tile_pool(name="lhs", bufs=4))
        work = ctx.enter_context(tc.tile_pool(name="work", bufs=3))
        pbig = ctx.enter_context(tc.tile_pool(name="pbig", bufs=2, space="PSUM"))
        psmall = ctx.enter_context(tc.tile_pool(name="psmall", bufs=2, space="PSUM"))

        b1t = const.tile([P, 16], F32, tag="b1t")
        nc.sync.dma_start(out=b1t[:], in_=b1_pc[:, :])
        b2st = const.tile([R, 1], F32, tag="b2st")
        nc.sync.dma_start(out=b2st[:], in_=b2s_pc[:, :])
        b2w1t = const.tile([R, P], F32, tag="b2w1t")
        nc.sync.dma_start(out=b2w1t[:], in_=b2w1[:, :])
        latTt = const.tile([P, B * N], F32, tag="latTt")
        nc.sync.dma_start(out=latTt[:], in_=latTd[0:P, :])
        bbct = const.tile([P, B * N], F32, tag="bbct")
        nc.sync.dma_start(out=bbct[:], in_=bbc[:, :])

        gT = [[persist.tile([P, N], BF16, tag=f"g{b}_{dm}", name=f"g{b}_{dm}") for dm in range(8)]
              for b in range(B)]
        gown = persist.tile([P, B * N], F32, tag="gown")
        hid = [persist.tile([P, B * N], BF16, tag=f"hid{m}", name=f"hid{m}") for m in range(16)]
        w2st = persist.tile([R, B * N], F32, tag="w2st")
        acc = persist.tile([P, B * N], F32, tag="acc")

        # Phase 1: gated scan
        for dm in range(8):
            ldt = work.tile([P, B * N], F32, tag="ldt", bufs=1, name="ldt")
            nc.sync.dma_start(out=ldt[:], in_=latTd[dm * P:(dm + 1) * P, :])
            for b in range(B):
                sl = slice(b * N, (b + 1) * N)
                if dm == 0:
                    nc.vector.tensor_tensor_scan(gown[:, sl], bbct[:, sl], ldt[:, sl],
                                                 0.0, mybir.AluOpType.mult,
                                                 mybir.AluOpType.add)
                    nc.scalar.activation(gT[b][0][:, :], gown[:, sl], AF.Copy)
                else:
                    nc.vector.tensor_tensor_scan(gT[b][dm][:, :], bbct[:, sl], ldt[:, sl],
                                                 0.0, mybir.AluOpType.mult,
                                                 mybir.AluOpType.add)

        # Phase 2: mm1 -> hid (gelu tanh-approx == x*sigmoid(1.5957691216*(x+0.044715x^3)))
        w2h = persist.tile([P, 8 * H], BF16, tag="w2h")
        for k in range(8):
            nc.sync.dma_start(out=w2h[:, k * H:(k + 1) * H],
                              in_=w1T[k * P:(k + 1) * P, :])
        for m in range(16):
            for b in range(B):
                ph = pbig.tile([P, N], F32, tag="big", name="ph")
                for k in range(8):
                    for jj in range(2):
                        nc.tensor.matmul(ph[:, jj * 512:(jj + 1) * 512],
                                         w2h[:, k * H + m * P:k * H + (m + 1) * P],
                                         gT[b][k][:, jj * 512:(jj + 1) * 512],
                                         start=(k == 0), stop=(k == 7))
                xg = work.tile([P, N], F32, tag="xg", bufs=2, name="xg")
                nc.scalar.activation(xg[:], ph[:], AF.Identity, bias=b1t[:, m:m + 1])
                ta = work.tile([P, N], F32, tag="tmpA", bufs=2, name="ta")
                nc.scalar.activation(ta[:], xg[:], AF.Square, scale=0.21146040470)
                tb = work.tile([P, N], F32, tag="tmpB", bufs=2, name="tb")
                nc.vector.tensor_mul(tb[:], ta[:], xg[:])
                ta2 = work.tile([P, N], F32, tag="tmpA", bufs=2, name="ta2")
                nc.vector.tensor_add(ta2[:], xg[:], tb[:])
                tb2 = work.tile([P, N], F32, tag="tmpB", bufs=2, name="tb2")
                nc.scalar.activation(tb2[:], ta2[:], AF.Sigmoid, scale=1.5957691216)
                nc.vector.tensor_mul(hid[m][:, b * N:(b + 1) * N], xg[:], tb2[:])

        # Phase 3: w2s
        for n in range(2):
            pw = pbig.tile([R, N], F32, tag="big", name="pw")
            for k in range(16):
                wt = lhs_pool.tile([P, R], BF16, tag="w2slhs", name="w2slhs")
                nc.sync.dma_start(out=wt[:], in_=W2sT[k * P:(k + 1) * P, :])
                for jj in range(2):
                    nc.tensor.matmul(pw[:, jj * 512:(jj + 1) * 512], wt[:],
                                     hid[k][:, n * N + jj * 512:n * N + (jj + 1) * 512],
                                     start=(k == 0), stop=(k == 15))
            nc.scalar.activation(w2st[:, n * N:(n + 1) * N], pw[:], AF.Identity,
                                 bias=b2st[:, 0:1])
            nc.sync.dma_start(out=w2s_dram[:, n * N:(n + 1) * N], in_=w2st[:, n * N:(n + 1) * N])

        # Phase 4: acc seed + mm2 + r-contraction
        for n in range(4):
            psd = psmall.tile([P, 512], F32, tag="small", name="psd")
            nc.tensor.matmul(psd[:], b2w1t[:], w2st[:, n * 512:(n + 1) * 512],
                             start=True, stop=True)
            nc.scalar.activation(acc[:, n * 512:(n + 1) * 512], psd[:], AF.Copy)

        for h in range(2):
            for kk in range(8):
                k = h * 8 + kk
                nc.sync.dma_start(out=w2h[:, kk * H:(kk + 1) * H],
                                  in_=w2T_shard[k * P:(k + 1) * P, :])
            for m in range(16):
                for n in range(2):
                    pm = pbig.tile([P, N], F32, tag="big", name="pm")
                    for kk in range(8):
                        k = h * 8 + kk
                        for jj in range(2):
                            nc.tensor.matmul(pm[:, jj * 512:(jj + 1) * 512],
                                             w2h[:, kk * H + m * P:kk * H + (m + 1) * P],
                                             hid[k][:, n * N + jj * 512:n * N + (jj + 1) * 512],
                                             start=(kk == 0), stop=(kk == 7))
                    wb = work.tile([P, N], F32, tag="tmpA", bufs=2, name="wb")
                    nc.sync.dma_start(out=wb[:], in_=w2s_dram[m:m + 1, n * N:(n + 1) * N]
                                      .to_broadcast([P, N]))
                    tmp = work.tile([P, N], F32, tag="tmpB", bufs=2, name="tmp")
                    nc.vector.tensor_mul(tmp[:], pm[:], wb[:])
                    nc.vector.tensor_add(acc[:, n * N:(n + 1) * N],
                                         acc[:, n * N:(n + 1) * N], tmp[:])

        # Phase 5: out = latT + gown * acc
        for n in range(2):
            sl = slice(n * N, (n + 1) * N)
            ctrl = work.tile([P, N], F32, tag="tmpA", bufs=2, name="ctrl")
            nc.vector.tensor_mul(ctrl[:], acc[:, sl], gown[:, sl])
            ot = work.tile([P, N], F32, tag="tmpB", bufs=2, name="ot")
            nc.vector.tensor_add(ot[:], ctrl[:], latTt[:, sl])
            nc.sync.dma_start(out=outT[:, sl], in_=ot[:])
    return nc


def _p2_finish(results):
    out = np.empty((B, N, D), np.float32)
    for c in range(8):
        o = np.asarray(results[c]["outT"])
        out[:, :, c * P:(c + 1) * P] = o.reshape(P, B, N).transpose(1, 2, 0)
    return out


# ----------------------------------------------------------------- kernel()
_cache = {}


def _get_programs():
    if "nc1" not in _cache:
        nc1 = bass.Bass()
        p1r_build(nc1)
        _cache["nc1"] = nc1
        nc2 = bacc.Bacc(None, target_bir_lowering=False)
        _p2_build(nc2)
        nc2.finalize()
        _cache["nc2"] = nc2
    return _cache["nc1"], _cache["nc2"]


def _get_maps(inputs):
    key = tuple(id(inputs[k]) for k in sorted(inputs))
    if _cache.get("maps_key") != key:
        _cache["maps1"] = [p1r_host_prep(inputs)] * 8
        _cache["maps2"] = [_p2_host_prep(inputs, c) for c in range(8)]
        _cache["maps_key"] = key
    return _cache["maps1"], _cache["maps2"]


LAST_EXEC_NS = [None, None]


def kernel(**inputs):
    nc1, nc2 = _get_programs()
    maps1, maps2s = _get_maps(inputs)
    r1 = run_bass_kernel_spmd(nc1, maps1, list(range(8)))
    beta = p1r_finish(r1.results)
    bmap = _p2_beta_prep(beta)
    maps2 = [dict(m, **bmap) for m in maps2s]
    r2 = run_bass_kernel_spmd(nc2, maps2, list(range(8)))
    LAST_EXEC_NS[0] = r1.exec_time_ns
    LAST_EXEC_NS[1] = r2.exec_time_ns
    return _p2_finish(r2.results)
